# revision 9
# baseline (speedup 1.0000x reference)
"""Trainium2 Bass kernel for causal attention with additive bias + key padding mask.

Problem: B=2, H=16, S=2048, D=128 (fp32), attn_bias [H,S,S], mask [B,1,1,S], offset=0.

Sharding: 32 (b,h) pairs across 8 cores; core c gets heads (2c, 2c+1) of both
batches, so every core sees the same mix of key-length caps.

Design (scores kept transposed s_T[j, q], per core):
  QK: bf16 matmul per key block into PSUM f32 (exact to ~1e-3).
  exp+bias, split per group by a static greedy balance:
    ACT path: pt = Exp(s) on ACT (fp16), then ptm = pt * eb on DVE or Pool
      (eb = exp(bias) fp16, masks folded as exact 0).
    DVE path: one fused op: ptm_bits = (s * A) + BB[j,i] -> uint16 with
      round-to-nearest + saturation; bits ARE fp16 exp(s + bias)
      (Schraudolph). BB = bias*A + B fp16, masked -> -30000 (saturates to 0).
  PV: fp16 matmul per block accumulating out_T[d,q] in PSUM.
  sums: per-block fp16 lane accumulators, DVE (sacc) / Pool (pacc) greedy;
  host reduces partitions. o_ps drains (PSUM->SBUF fp16) on ACT or DVE.
  Final normalization (outT / sums) and transpose on host.
"""

import os
from contextlib import ExitStack

import ml_dtypes
import numpy as np

_B, _H, _S, _D = 2, 16, 2048, 128
_NCORES = 8
_NH = (_B * _H) // _NCORES  # (b,h) pairs per core = 4
_QCH = 512
_NQC = _S // _QCH
_G = 3  # key blocks per exp group
_PIPE = 4  # group-level pipeline lag between QK/exp and PV/sums

# fp16 Schraudolph exp: bits16(exp(x)) ~= x*A + B (round-to-nearest, sat)
_SA = 1024.0 / float(np.log(2.0))
_SB = 15360.0 - 1024.0 * 0.0573
_BB_MASK = -30000.0  # forces saturate-to-0 bits for masked entries

# engine rates (ns per 128-lane column) for the static greedy balance
_R_ACT = 1.0 / 1.2
_R_DVE_EXP = 1.0 / 0.96           # f32 psum input -> no 2x
_R_DVE_MUL = 0.5 / 0.96           # fp16 all-SBUF 2x
_R_POOL_MUL = 1.0 / (1.2 * 0.42)  # Pool tensor_tensor efficiency 0.42
_OVH = 250.0

_PROG_CACHE = {}
LAST_RESULTS = None
LAST_IN_MAPS = None
LAST_BUILD_KW = None


def _schedule(caps, G=_G):
    """Static per-core schedule: groups, fp16 blob offsets, and greedy engine
    assignment: per group exp path (ACT+mult vs fused DVE), per-group mult
    engine (DVE/Pool), per-block sums engine (DVE/Pool), per-chunk drain
    engine (ACT/DVE)."""
    NH, QCH, NQC = _NH, _QCH, _NQC
    plan = []
    off = 0
    act_t = dve_t = pool_t = 0.0
    for h, cap in enumerate(caps):
        hplan = []
        for qc in range(NQC):
            q_end = (qc + 1) * QCH
            jb_hi = min((q_end + 127) // 128, cap)
            groups = []
            g0 = 0
            while g0 < jb_hi:
                Gp = min(G, jb_hi - g0)
                qo = max(0, g0 * 128 - qc * QCH)
                nq = QCH - qo
                boff = off
                off += Gp * nq
                C = Gp * nq
                # option A: ACT exp + fp16 eb mult on DVE or Pool
                act_t2 = act_t + C * _R_ACT + _OVH
                if dve_t + C * _R_DVE_MUL <= pool_t + C * _R_POOL_MUL:
                    meng, m_t2 = "dve", dve_t + C * _R_DVE_MUL + _OVH
                else:
                    meng, m_t2 = "pool", pool_t + C * _R_POOL_MUL + _OVH
                finA = max(act_t2, m_t2)
                # option B: fused Schraudolph on DVE
                finB = dve_t + C * _R_DVE_EXP + _OVH
                if finA <= finB:
                    exp = "act"
                    act_t = act_t2
                    if meng == "dve":
                        dve_t = m_t2
                    else:
                        pool_t = m_t2
                else:
                    exp = "dve"
                    meng = None
                    dve_t = finB
                groups.append(
                    {"g0": g0, "Gp": Gp, "qo": qo, "nq": nq, "boff": boff,
                     "exp": exp, "mult": meng}
                )
                g0 += Gp
            sums_eng = []
            for jb in range(jb_hi):
                qb = max(0, jb * 128 - qc * QCH)
                nqe = QCH - qb
                if dve_t + nqe * _R_DVE_MUL <= pool_t + nqe * _R_POOL_MUL:
                    sums_eng.append("dve")
                    dve_t += nqe * _R_DVE_MUL + _OVH
                else:
                    sums_eng.append("pool")
                    pool_t += nqe * _R_POOL_MUL + _OVH
            if act_t + QCH * _R_ACT + 150 <= dve_t + QCH * _R_DVE_EXP:
                drain = "act"
                act_t += QCH * _R_ACT + _OVH + 150
            else:
                drain = "dve"
                dve_t += QCH * _R_DVE_EXP + _OVH
            hplan.append(
                {"jb_hi": jb_hi, "groups": groups, "sums": sums_eng,
                 "drain": drain}
            )
        plan.append(hplan)
    return plan, off


def _build_program(caps, repeat=1, G=_G, pipe=_PIPE):
    import contextlib

    import concourse.bacc as bacc
    import concourse.mybir as mybir
    import concourse.tile as tile

    caps = tuple(caps)
    NH, S, D, QCH, NQC = _NH, _S, _D, _QCH, _NQC
    kof = [sum(caps[:h]) for h in range(NH + 1)]
    KBT = kof[NH]
    f32 = mybir.dt.float32
    f16 = mybir.dt.float16
    bf16 = mybir.dt.bfloat16
    u16 = mybir.dt.uint16

    plan, BLOB = _schedule(caps, G)

    nc = bacc.Bacc("TRN2", target_bir_lowering=False, debug=False)

    kt_d = nc.dram_tensor("kt", [128, KBT * 128], bf16, kind="ExternalInput").ap()
    qt_d = nc.dram_tensor("qt", [NH, 128, S], bf16, kind="ExternalInput").ap()
    blob_d = nc.dram_tensor("blob", [128, BLOB], f16, kind="ExternalInput").ap()
    v_d = nc.dram_tensor("v", [128, KBT, D], f16, kind="ExternalInput").ap()
    outT_d = nc.dram_tensor("outT", [NH, D, S], f16, kind="ExternalOutput").ap()
    sacc_d = nc.dram_tensor("sacc", [NH, NQC, 128, QCH], f16, kind="ExternalOutput").ap()
    pacc_d = nc.dram_tensor("pacc", [NH, NQC, 128, QCH], f16, kind="ExternalOutput").ap()

    with tile.TileContext(nc) as tc, ExitStack() as ctx:
        const = ctx.enter_context(tc.tile_pool(name="const", bufs=1))
        ptp = ctx.enter_context(tc.tile_pool(name="ptp", bufs=2))
        ptmp = ctx.enter_context(tc.tile_pool(name="ptmp", bufs=pipe + 1))
        saccp = ctx.enter_context(tc.tile_pool(name="saccp", bufs=2))
        paccp = ctx.enter_context(tc.tile_pool(name="paccp", bufs=2))
        obp = ctx.enter_context(tc.tile_pool(name="obp", bufs=2))
        psum_s = ctx.enter_context(tc.tile_pool(name="psum_s", bufs=2, space="PSUM"))
        psum_o = ctx.enter_context(tc.tile_pool(name="psum_o", bufs=2, space="PSUM"))

        kt_sb = const.tile([128, KBT * 128], bf16)
        qt_sb = const.tile([128, NH, S], bf16)
        blob_sb = const.tile([128, BLOB], f16)
        v_sb = const.tile([128, KBT, D], f16)
        nc.sync.dma_start(out=kt_sb[:], in_=kt_d[:])
        nc.sync.dma_start(out=v_sb[:], in_=v_d[:])
        for h in range(NH):
            nc.sync.dma_start(out=qt_sb[:, h, :], in_=qt_d[h])
        nch = 8
        step = (BLOB + nch - 1) // nch
        for i in range(nch):
            lo = i * step
            hi = min(BLOB, lo + step)
            if lo < hi:
                (nc.sync if i % 2 else nc.gpsimd).dma_start(
                    out=blob_sb[:, lo:hi], in_=blob_d[:, lo:hi]
                )

        loop_cm = tc.For_i(0, repeat, 1) if repeat > 1 else contextlib.nullcontext()
        with loop_cm:
            stages = []
            chunk_ctx = {}
            for h in range(NH):
                for qc in range(NQC):
                    chunk = plan[h][qc]
                    ck = {
                        "h": h, "qc": qc, "jb_hi": chunk["jb_hi"],
                        "sums": chunk["sums"], "drain": chunk["drain"],
                        "o_ps": None, "sacc": None, "pacc": None,
                    }
                    chunk_ctx[(h, qc)] = ck
                    ngrp = len(chunk["groups"])
                    for idx, g in enumerate(chunk["groups"]):
                        stages.append((ck, g, idx == ngrp - 1))

            stash = {}
            for gi in range(len(stages) + pipe):
                if gi < len(stages):
                    ck, g, _last = stages[gi]
                    h, qc = ck["h"], ck["qc"]
                    g0, Gp, qo, nq = g["g0"], g["Gp"], g["qo"], g["nq"]
                    s3 = psum_s.tile([128, G, QCH], f32)
                    for i in range(Gp):
                        jb = g0 + i
                        nc.tensor.matmul(
                            s3[:, i, qo:],
                            lhsT=kt_sb[
                                :, (kof[h] + jb) * 128 : (kof[h] + jb + 1) * 128
                            ],
                            rhs=qt_sb[:, h, qc * QCH + qo : (qc + 1) * QCH],
                            start=True,
                            stop=True,
                        )
                    bbv = blob_sb[:, g["boff"] : g["boff"] + Gp * nq].rearrange(
                        "p (g n) -> p g n", g=Gp
                    )
                    ptm3 = ptmp.tile([128, G, QCH], f16)
                    if g["exp"] == "act":
                        pt3 = ptp.tile([128, G, QCH], f16)
                        nc.scalar.activation(
                            pt3[:, :Gp, qo:],
                            s3[:, :Gp, qo:],
                            mybir.ActivationFunctionType.Exp,
                        )
                        api = nc.vector if g["mult"] == "dve" else nc.gpsimd
                        api.tensor_mul(ptm3[:, :Gp, qo:], pt3[:, :Gp, qo:], bbv)
                    else:
                        nc.vector.scalar_tensor_tensor(
                            ptm3[:, :Gp, qo:].bitcast(u16),
                            s3[:, :Gp, qo:],
                            _SA,
                            bbv,
                            op0=mybir.AluOpType.mult,
                            op1=mybir.AluOpType.add,
                        )
                    stash[gi] = ptm3
                bi = gi - pipe
                if 0 <= bi < len(stages):
                    ck, g, last = stages[bi]
                    h, qc = ck["h"], ck["qc"]
                    jb_hi = ck["jb_hi"]
                    g0, Gp, qo = g["g0"], g["Gp"], g["qo"]
                    ptm3 = stash.pop(bi)
                    if ck["o_ps"] is None:
                        ck["o_ps"] = psum_o.tile([128, QCH], f32, name="o_ps")
                    o_ps = ck["o_ps"]
                    for i in range(Gp):
                        jb = g0 + i
                        qb = max(qo, jb * 128 - qc * QCH)
                        nc.tensor.matmul(
                            o_ps[:, qb:],
                            lhsT=v_sb[:, kof[h] + jb, :],
                            rhs=ptm3[:, i, qb:],
                            start=(jb == 0),
                            stop=(jb == jb_hi - 1),
                        )
                        eng = ck["sums"][jb]
                        api = nc.vector if eng == "dve" else nc.gpsimd
                        key = "sacc" if eng == "dve" else "pacc"
                        acc = ck[key]
                        if acc is None:
                            pool = saccp if eng == "dve" else paccp
                            acc = pool.tile([128, QCH], f16, name=key)
                            ck[key] = acc
                            api.tensor_copy(acc[:, qb:], ptm3[:, i, qb:])
                            if qb > 0:
                                api.memset(acc[:, :qb], 0.0)
                        else:
                            api.tensor_add(acc[:, qb:], acc[:, qb:], ptm3[:, i, qb:])
                    if last:
                        ob = obp.tile([128, QCH], f16)
                        if ck["drain"] == "act":
                            nc.scalar.copy(ob[:], o_ps[:])
                        else:
                            nc.vector.tensor_copy(ob[:], o_ps[:])
                        nc.gpsimd.dma_start(
                            out=outT_d[h, :, qc * QCH : (qc + 1) * QCH], in_=ob[:]
                        )
                        if ck["sacc"] is not None:
                            nc.gpsimd.dma_start(out=sacc_d[h, qc], in_=ck["sacc"][:])
                        if ck["pacc"] is not None:
                            nc.sync.dma_start(out=pacc_d[h, qc], in_=ck["pacc"][:])

    nc.compile()
    return nc


def _run_multicore(in_maps, caps):
    global LAST_RESULTS, LAST_IN_MAPS, LAST_BUILD_KW
    from concourse.bass_utils import run_bass_kernel_spmd

    key = (tuple(caps), _G, _PIPE)
    if key not in _PROG_CACHE:
        _PROG_CACHE[key] = _build_program(caps)
    nc = _PROG_CACHE[key]
    LAST_IN_MAPS = in_maps
    LAST_BUILD_KW = {"caps": tuple(caps), "G": _G, "pipe": _PIPE}
    res = run_bass_kernel_spmd(nc, in_maps, core_ids=list(range(len(in_maps))))
    LAST_RESULTS = res
    return res.results


def kernel(q, k, v, mask, attn_bias, offset):
    B, H, S, D = _B, _H, _S, _D
    q = np.asarray(q, dtype=np.float32)
    k = np.asarray(k, dtype=np.float32)
    v = np.asarray(v, dtype=np.float32)
    mask = np.asarray(mask).astype(bool)
    attn_bias = np.asarray(attn_bias, dtype=np.float32)
    off = int(np.asarray(offset))

    scale = np.float32(D**-0.5)
    valid = mask[:, 0, 0, :]  # [B, S]

    caps_b = []
    for b in range(B):
        idx = np.nonzero(valid[b])[0]
        lv = (int(idx[-1]) + 1) if len(idx) else 1
        caps_b.append(max(1, (lv + 127) // 128))

    # biasT[h][j, i] = attn_bias[h, i, j]; keep[j, i] = causal-valid
    jj = np.arange(S)[:, None]
    ii = np.arange(S)[None, :]
    keep_causal = jj < (ii + 1 - off)  # [j, i]
    biasT = attn_bias.transpose(0, 2, 1)

    core_pairs = [
        [(0, 2 * c), (0, 2 * c + 1), (1, 2 * c), (1, 2 * c + 1)]
        for c in range(_NCORES)
    ]
    caps = tuple(caps_b[b] for (b, _h) in core_pairs[0])
    plan, BLOB = _schedule(caps)
    kof = [sum(caps[:h]) for h in range(_NH + 1)]
    KBT = kof[_NH]

    bf = ml_dtypes.bfloat16
    in_maps = []
    for c in range(_NCORES):
        pairs = core_pairs[c]
        kt = np.concatenate(
            [(k[b, h][: caps_b[b] * 128] * scale).T for (b, h) in pairs], axis=1
        ).astype(bf)
        qt = np.stack([q[b, h].T for (b, h) in pairs]).astype(bf)
        vv = np.concatenate(
            [
                v[b, h][: caps_b[b] * 128].reshape(caps_b[b], 128, D).transpose(1, 0, 2)
                for (b, h) in pairs
            ],
            axis=1,
        ).astype(np.float16)
        blob = np.zeros((128, BLOB), dtype=np.float16)
        for hh, (b, h) in enumerate(pairs):
            keep_bh = keep_causal & valid[b][:, None]  # [j, i]
            for qc in range(_NQC):
                for g in plan[hh][qc]["groups"]:
                    qo, nq = g["qo"], g["nq"]
                    o = g["boff"]
                    for i in range(g["Gp"]):
                        jb = g["g0"] + i
                        js = slice(jb * 128, (jb + 1) * 128)
                        is_ = slice(qc * _QCH + qo, (qc + 1) * _QCH)
                        bblk = biasT[h][js, is_]
                        kblk = keep_bh[js, is_]
                        if g["exp"] == "act":
                            ent = np.where(kblk, np.exp(bblk), 0.0)
                        else:
                            ent = np.where(kblk, bblk * _SA + _SB, _BB_MASK)
                        blob[:, o + i * nq : o + (i + 1) * nq] = ent.astype(
                            np.float16
                        )
        in_maps.append(
            {
                "kt": np.ascontiguousarray(kt),
                "qt": np.ascontiguousarray(qt),
                "blob": blob,
                "v": np.ascontiguousarray(vv),
            }
        )

    results = _run_multicore(in_maps, caps)

    out = np.empty((B, H, S, D), dtype=np.float32)
    for c in range(_NCORES):
        res = results[c]
        outT = res["outT"].astype(np.float32)  # [NH, D, S]
        sums = (
            res["sacc"].astype(np.float32).sum(axis=2)
            + res["pacc"].astype(np.float32).sum(axis=2)
        ).reshape(_NH, S)
        for i, (b, h) in enumerate(core_pairs[c]):
            out[b, h] = (outT[i] / sums[i][None, :]).T
    return out


# revision 21
# speedup vs baseline: 1.7744x; 1.7744x over previous
"""Trainium2 Bass kernel for causal attention with additive bias + key padding mask.

Problem: B=2, H=16, S=2048, D=128 (fp32), attn_bias [H,S,S], mask [B,1,1,S], offset=0.

Sharding: 32 (b,h) pairs across 8 cores, mixed-batch: core c gets heads (2c, 2c+1)
of BOTH batches so every core sees the same mix of key-length caps.

Device math (per core, scores kept transposed: s_T[j, q], so no transposes):
  s_T = KT_blk^T @ QT_chunk          (PE, bf16, psum f32)
  pt  = exp(s_T)                     (ACT, psum -> sbuf fp16, 3-block grouped)
  ptm = pt * ebias                   (DVE fp16 2x; ebias = exp(bias) with causal +
                                      key-pad masks folded as exact zeros, fp16,
                                      packed ragged and fully SBUF-resident)
  out_T[d,q] += V_blk^T @ ptm        (PE, fp16 rhs)
  sums hybrid: leading fs blocks per chunk via PE one-hot-ones matmul into a
        per-head [NQC,512] psum (one DVE drain per head); remaining blocks
        accumulated into a fp16 sacc tile on DVE (copy-init). Host combines.
The o_ps drain (psum -> bf16 staging) alternates between the ACT and DVE
engines per chunk to balance queue load; the B-stage trails the A-stage by 4
groups globally so the pipeline never drains at chunk/head boundaries.
Final normalization (outT / sums) and transpose on host.
"""

import os
from contextlib import ExitStack

import ml_dtypes
import numpy as np

_B, _H, _S, _D = 2, 16, 2048, 128
_NCORES = 8
_NH = (_B * _H) // _NCORES  # heads per core = 4
_QCH = 512
_NQC = _S // _QCH
_G = 3  # blocks per exp/mult group
_FSX = 0.25  # fraction of each chunk's key blocks whose sums go via PE
_PSX = 0  # (unused) Pool sums share
_PIPE = 4  # group-level software pipeline lag for PV/sums emission

_PROG_CACHE = {}
LAST_RESULTS = None
LAST_IN_MAPS = None
LAST_BUILD_KW = None


def _schedule(caps, G=_G, fsx=_FSX, psx=_PSX):
    """Static per-core schedule; shared by host packing and device build."""
    plan = []
    off = 0
    for h, cap in enumerate(caps):
        hplan = []
        for qc in range(_NQC):
            q_end = (qc + 1) * _QCH
            jb_hi = min((q_end + 127) // 128, cap)
            groups = []
            g0 = 0
            while g0 < jb_hi:
                Gp = min(G, jb_hi - g0)
                qo = max(0, g0 * 128 - qc * _QCH)
                nq = _QCH - qo
                groups.append({"g0": g0, "Gp": Gp, "qo": qo, "nq": nq, "off": off})
                off += Gp * nq
                g0 += Gp
            # fs rounded to a multiple of G: no group straddles the one-hot
            # boundary, so gsum sums need no memset (first group copy-inits)
            fs = G * int(round(fsx * jb_hi / G)) if fsx > 0 else 0
            fs = min(fs, jb_hi)
            hplan.append({"jb_hi": jb_hi, "fs": fs, "groups": groups})
        plan.append(hplan)
    return plan, off


def _build_program(caps, repeat=1, G=_G, fsx=_FSX, psx=_PSX, pipe=_PIPE, drop="", unroll=False, gsum=_GSUM):
    import contextlib

    import concourse.bacc as bacc
    import concourse.mybir as mybir
    import concourse.tile as tile

    caps = tuple(caps)
    NH, S, D, QCH, NQC = _NH, _S, _D, _QCH, _NQC
    NB = S // 128
    kof = [sum(caps[:h]) for h in range(NH + 1)]  # ragged key-block offsets
    f32 = mybir.dt.float32
    f16 = mybir.dt.float16
    bf16 = mybir.dt.bfloat16

    plan, CF = _schedule(caps, G, fsx, psx)
    use_r = any(ch["fs"] > 0 for hp in plan for ch in hp)

    nc = bacc.Bacc("TRN2", target_bir_lowering=False, debug=False)

    KBT = kof[NH]  # total key blocks across heads
    kt_d = nc.dram_tensor("kt", [128, KBT * 128], bf16, kind="ExternalInput").ap()
    qt_d = nc.dram_tensor("qt", [NH, 128, S], bf16, kind="ExternalInput").ap()
    v_d = nc.dram_tensor("v", [128, KBT, D], f16, kind="ExternalInput").ap()
    eb_d = nc.dram_tensor("eb", [128, CF], f16, kind="ExternalInput").ap()
    outT_d = nc.dram_tensor("outT", [NH, D, S], bf16, kind="ExternalOutput").ap()
    SW = G if gsum else 1  # sacc slot width
    sacc_d = nc.dram_tensor(
        "sacc", [NH, NQC, 128, SW, QCH], f16, kind="ExternalOutput"
    ).ap()
    if use_r:
        r_d = nc.dram_tensor("r", [NH, NQC, QCH], f32, kind="ExternalOutput").ap()
        oh_d = nc.dram_tensor("oh", [128, NQC * NQC], f16, kind="ExternalInput").ap()

    with tile.TileContext(nc) as tc, ExitStack() as ctx:
        const = ctx.enter_context(tc.tile_pool(name="const", bufs=1))
        ptp = ctx.enter_context(tc.tile_pool(name="ptp", bufs=2))
        ptmp = ctx.enter_context(tc.tile_pool(name="ptmp", bufs=pipe + 1 + (1 if "ptm6" in drop else 0)))
        saccp = ctx.enter_context(tc.tile_pool(name="saccp", bufs=2))
        obp = ctx.enter_context(tc.tile_pool(name="obp", bufs=1 if pipe >= 5 else 2))
        rbp = ctx.enter_context(tc.tile_pool(name="rbp", bufs=1))
        psum_s = ctx.enter_context(tc.tile_pool(name="psum_s", bufs=3 if G == 2 else 2, space="PSUM"))
        psum_o = ctx.enter_context(tc.tile_pool(name="psum_o", bufs=1, space="PSUM"))
        if use_r:
            psum_r = ctx.enter_context(tc.tile_pool(name="psum_r", bufs=1, space="PSUM"))

        # one-hot "ones" weights (host-provided): oh[:, qc*NQC+qc] = 1, so chunk
        # qc's sums land in psum partition row qc of a per-head [NQC, QCH] accum
        if use_r:
            oh_sb = const.tile([128, NQC * NQC], f16)
            nc.sync.dma_start(out=oh_sb[:], in_=oh_d[:])
            ohs = [oh_sb[:, qc * NQC : (qc + 1) * NQC] for qc in range(NQC)]

        kt_sb = const.tile([128, KBT * 128], bf16)
        qt_sb = const.tile([128, NH, S], bf16)
        v_sb = const.tile([128, KBT, D], f16)
        eb_sb = const.tile([128, CF], f16)
        nc.sync.dma_start(out=kt_sb[:], in_=kt_d[:])
        nc.sync.dma_start(out=v_sb[:], in_=v_d[:])
        for h in range(NH):
            nc.sync.dma_start(out=qt_sb[:, h, :], in_=qt_d[h])
        nch = 8
        step = (CF + nch - 1) // nch
        for i in range(nch):
            lo = i * step
            hi = min(CF, lo + step)
            if lo < hi:
                (nc.sync if i % 2 else nc.gpsimd).dma_start(
                    out=eb_sb[:, lo:hi], in_=eb_d[:, lo:hi]
                )

        loop_cm = (
            tc.For_i(0, repeat, 1)
            if (repeat > 1 and not unroll)
            else contextlib.nullcontext()
        )
        with loop_cm:
          for _rep in range(repeat if unroll else 1):
            # flat stage list: one entry per (head, chunk, group); the B-stage
            # (PV + sums) trails the A-stage (QK + exp + mult) by `pipe` slots
            # globally, so the cross-engine pipeline never drains at chunk or
            # head boundaries.
            stages = []
            head_ctx = {}
            chunk_ctx = {}
            for h in range(NH):
                fsqc = [q for q in range(NQC) if plan[h][q]["fs"] > 0]
                head_ctx[h] = {
                    "r_ps": None,
                    "fs_first": fsqc[0] if fsqc else -1,
                    "fs_last": fsqc[-1] if fsqc else -1,
                }
                for qc in range(NQC):
                    chunk = plan[h][qc]
                    ck = {
                        "h": h,
                        "qc": qc,
                        "jb_hi": chunk["jb_hi"],
                        "fs": chunk["fs"],
                        "o_ps": None,
                        "sacc": None,
                    }
                    chunk_ctx[(h, qc)] = ck
                    ngrp = len(chunk["groups"])
                    for idx, g in enumerate(chunk["groups"]):
                        stages.append((ck, g, idx == ngrp - 1))

            stash = {}
            for gi in range(len(stages) + pipe):
                if gi < len(stages):
                    ck, g, _last = stages[gi]
                    h, qc = ck["h"], ck["qc"]
                    g0, Gp, qo, nq, off = (
                        g["g0"], g["Gp"], g["qo"], g["nq"], g["off"],
                    )
                    s3 = psum_s.tile([128, G, QCH], f32)
                    for i in range(Gp):
                        jb = g0 + i
                        nc.tensor.matmul(
                            s3[:, i, qo:],
                            lhsT=kt_sb[
                                :, (kof[h] + jb) * 128 : (kof[h] + jb + 1) * 128
                            ],
                            rhs=qt_sb[:, h, qc * QCH + qo : (qc + 1) * QCH],
                            start=True,
                            stop=True,
                        )
                    pt3 = ptp.tile([128, G, QCH], f16)
                    nc.scalar.activation(
                        pt3[:, :Gp, qo:],
                        s3[:, :Gp, qo:],
                        mybir.ActivationFunctionType.Exp,
                    )
                    ptm3 = ptmp.tile([128, G, QCH], f16)
                    ebv = eb_sb[:, off : off + Gp * nq].rearrange(
                        "p (g n) -> p g n", g=Gp
                    )
                    if "mult" not in drop:
                        nc.vector.tensor_mul(ptm3[:, :Gp, qo:], pt3[:, :Gp, qo:], ebv)
                    else:
                        ptm3 = pt3
                    stash[gi] = ptm3
                bi = gi - pipe
                if 0 <= bi < len(stages):
                    ck, g, last = stages[bi]
                    h, qc = ck["h"], ck["qc"]
                    jb_hi, fs = ck["jb_hi"], ck["fs"]
                    g0, Gp, qo = g["g0"], g["Gp"], g["qo"]
                    ptm3 = stash.pop(bi)
                    if ck["o_ps"] is None:
                        ck["o_ps"] = psum_o.tile([128, QCH], f32, name="o_ps")
                        if fs < jb_hi:
                            ck["sacc"] = saccp.tile([128, SW, QCH], f16, name="sacc")
                            if gsum:
                                ck["sacc_init"] = False
                                if min(G, jb_hi - fs) < G:
                                    # slots never touched by the copy-init
                                    nc.gpsimd.memset(
                                        ck["sacc"][:, min(G, jb_hi - fs):, :], 0.0
                                    )
                    if fs > 0 and head_ctx[h]["r_ps"] is None:
                        head_ctx[h]["r_ps"] = psum_r.tile(
                            [NQC, QCH], f32, name="r_ps"
                        )
                    o_ps, sacc = ck["o_ps"], ck["sacc"]
                    r_ps = head_ctx[h]["r_ps"]
                    for i in range(Gp):
                        jb = g0 + i
                        qb = max(qo, jb * 128 - qc * QCH)  # per-block trim
                        if "pv" not in drop:
                            nc.tensor.matmul(
                                o_ps[:, qb:],
                                lhsT=v_sb[:, kof[h] + jb, :],
                                rhs=ptm3[:, i, qb:],
                                start=(jb == 0),
                                stop=(jb == jb_hi - 1),
                            )
                        elif jb == 0:
                            nc.tensor.matmul(
                                o_ps[:, :],
                                lhsT=v_sb[:, kof[h] + jb, :],
                                rhs=ptm3[:, i, :],
                                start=True,
                                stop=True,
                            )
                        if jb < fs:
                            nc.tensor.matmul(
                                r_ps[:, qb:],
                                lhsT=ohs[qc],
                                rhs=ptm3[:, i, qb:],
                                start=(qc == head_ctx[h]["fs_first"] and jb == 0),
                                stop=(qc == head_ctx[h]["fs_last"] and jb == fs - 1),
                                skip_group_check=True,
                            )
                        elif "sacc" in drop:
                            pass
                        elif gsum:
                            pass  # fs is a multiple of G: no straddle groups
                        elif jb == fs:
                            nc.vector.tensor_copy(sacc[:, 0, qb:], ptm3[:, i, qb:])
                            if qb > 0:
                                nc.gpsimd.memset(sacc[:, 0, :qb], 0.0)
                        else:
                            nc.vector.tensor_add(
                                sacc[:, 0, qb:], sacc[:, 0, qb:], ptm3[:, i, qb:]
                            )
                    if gsum and g0 >= fs and "sacc" not in drop:
                        # whole group accumulated in one DVE op (slot per block);
                        # first group per chunk copy-inits (qo == 0 there)
                        if not ck["sacc_init"]:
                            nc.vector.tensor_copy(
                                sacc[:, :Gp, qo:], ptm3[:, :Gp, qo:]
                            )
                            ck["sacc_init"] = True
                        else:
                            nc.vector.tensor_add(
                                sacc[:, :Gp, qo:], sacc[:, :Gp, qo:], ptm3[:, :Gp, qo:]
                            )
                    if last:
                        ob = obp.tile([128, QCH], bf16)
                        if "obdve" in drop:
                            nc.vector.tensor_copy(ob[:], o_ps[:])
                        elif "obact" not in drop and (h * NQC + qc) % 2 == 0:
                            nc.vector.tensor_copy(ob[:], o_ps[:])
                        else:
                            nc.scalar.copy(ob[:], o_ps[:])
                        nc.gpsimd.dma_start(
                            out=outT_d[h, :, qc * QCH : (qc + 1) * QCH], in_=ob[:]
                        )
                        if sacc is not None and "sacc" not in drop:
                            nc.gpsimd.dma_start(out=sacc_d[h, qc], in_=sacc[:])
                        if fs > 0 and qc == NQC - 1:
                            rb_h = rbp.tile([NQC, QCH], f32)
                            nc.vector.tensor_copy(rb_h[:], r_ps[:])
                            nc.sync.dma_start(out=r_d[h], in_=rb_h[:])
                            head_ctx[h]["r_ps"] = None

    nc.compile()
    return nc


def _pack_ebias(eb_masked, caps, G=_G, fsx=_FSX, psx=_PSX):
    """eb_masked: [NH, S(j), S(q)] f32 (exp(bias) with masks folded as 0).
    Returns [128, CF] fp16 ragged-packed per the schedule."""
    plan, CF = _schedule(caps, G, fsx, psx)
    out = np.zeros((128, CF), dtype=np.float16)
    for h, hplan in enumerate(plan):
        for qc, chunk in enumerate(hplan):
            for g in chunk["groups"]:
                g0, Gp, qo, nq, off = g["g0"], g["Gp"], g["qo"], g["nq"], g["off"]
                for i in range(Gp):
                    jb = g0 + i
                    blk = eb_masked[
                        h,
                        jb * 128 : (jb + 1) * 128,
                        qc * _QCH + qo : (qc + 1) * _QCH,
                    ]
                    out[:, off + i * nq : off + (i + 1) * nq] = blk.astype(np.float16)
    return out


def _run_multicore(in_maps, caps):
    global LAST_RESULTS, LAST_IN_MAPS, LAST_BUILD_KW
    from concourse.bass_utils import run_bass_kernel_spmd

    key = (tuple(caps), _G, _FSX, _PSX, _PIPE, _GSUM)
    if key not in _PROG_CACHE:
        _PROG_CACHE[key] = _build_program(caps)
    nc = _PROG_CACHE[key]
    LAST_IN_MAPS = in_maps
    LAST_BUILD_KW = {"caps": tuple(caps), "G": _G, "fsx": _FSX, "psx": _PSX, "pipe": _PIPE, "gsum": _GSUM}
    res = run_bass_kernel_spmd(nc, in_maps, core_ids=list(range(len(in_maps))))
    LAST_RESULTS = res
    return res.results


def kernel(q, k, v, mask, attn_bias, offset):
    B, H, S, D = _B, _H, _S, _D
    q = np.asarray(q, dtype=np.float32)
    k = np.asarray(k, dtype=np.float32)
    v = np.asarray(v, dtype=np.float32)
    mask = np.asarray(mask).astype(bool)
    attn_bias = np.asarray(attn_bias, dtype=np.float32)
    off = int(np.asarray(offset))

    scale = np.float32(D**-0.5)
    valid = mask[:, 0, 0, :]  # [B, S]

    caps_b = []
    for b in range(B):
        idx = np.nonzero(valid[b])[0]
        lv = (int(idx[-1]) + 1) if len(idx) else 1
        caps_b.append(max(1, (lv + 127) // 128))

    # ebias[h][j, i] = exp(attn_bias[h, i, j]), causal mask j >= i+1-off -> 0
    jj = np.arange(S)[:, None]
    ii = np.arange(S)[None, :]
    keep_causal = (jj < (ii + 1 - off)).astype(np.float32)  # [j, i]
    ebias = np.exp(attn_bias).transpose(0, 2, 1) * keep_causal[None]

    core_pairs = [
        [(0, 2 * c), (0, 2 * c + 1), (1, 2 * c), (1, 2 * c + 1)] for c in range(_NCORES)
    ]
    caps = tuple(caps_b[b] for (b, _h) in core_pairs[0])

    NB = S // 128
    bf = ml_dtypes.bfloat16
    in_maps = []
    for c in range(_NCORES):
        pairs = core_pairs[c]
        kt = np.concatenate(
            [(k[b, h][: caps_b[b] * 128] * scale).T for (b, h) in pairs], axis=1
        ).astype(bf)
        qt = np.stack([q[b, h].T for (b, h) in pairs]).astype(bf)
        vv = np.concatenate(
            [
                v[b, h][: caps_b[b] * 128].reshape(caps_b[b], 128, D).transpose(1, 0, 2)
                for (b, h) in pairs
            ],
            axis=1,
        ).astype(np.float16)
        ebm = np.stack(
            [ebias[h] * valid[b][:, None].astype(np.float32) for (b, h) in pairs]
        )
        oh = np.zeros((128, _NQC * _NQC), np.float16)
        for qc in range(_NQC):
            oh[:, qc * _NQC + qc] = 1.0
        in_maps.append(
            {
                "kt": np.ascontiguousarray(kt),
                "qt": np.ascontiguousarray(qt),
                "v": np.ascontiguousarray(vv),
                "eb": _pack_ebias(ebm, caps),
                "oh": oh,
            }
        )

    results = _run_multicore(in_maps, caps)

    out = np.empty((B, H, S, D), dtype=np.float32)
    for c in range(_NCORES):
        res = results[c]
        outT = res["outT"].astype(np.float32)  # [NH, D, S]
        sums = res["sacc"].astype(np.float32).sum(axis=(2, 3)).reshape(_NH, S)
        if "r" in res:
            sums = sums + res["r"].reshape(_NH, S)
        for i, (b, h) in enumerate(core_pairs[c]):
            out[b, h] = (outT[i] / sums[i][None, :]).T
    return out



# revision 22
# speedup vs baseline: 1.9323x; 1.0890x over previous
"""Trainium2 Bass kernel for causal attention with additive bias + key padding mask.

Problem: B=2, H=16, S=2048, D=128 (fp32), attn_bias [H,S,S], mask [B,1,1,S], offset=0.

Sharding: 32 (b,h) pairs across 8 cores, mixed-batch: core c gets heads (2c, 2c+1)
of BOTH batches so every core sees the same mix of key-length caps.

Device math (per core, scores kept transposed: s_T[j, q], so no transposes):
  s_T = KT_blk^T @ QT_chunk          (PE, bf16, psum f32)
  pt  = exp(s_T)                     (ACT, psum -> sbuf fp16, 3-block grouped)
  ptm = pt * ebias                   (DVE fp16 2x; ebias = exp(bias) with causal +
                                      key-pad masks folded as exact zeros, fp16,
                                      packed ragged and fully SBUF-resident)
  out_T[d,q] += V_blk^T @ ptm        (PE, fp16 rhs)
  sums hybrid: leading fs blocks per chunk via PE one-hot-ones matmul into a
        per-head [NQC,512] psum (one DVE drain per head); remaining blocks
        accumulated into a fp16 sacc tile on DVE (copy-init). Host combines.
The o_ps drain (psum -> bf16 staging) alternates between the ACT and DVE
engines per chunk to balance queue load; the B-stage trails the A-stage by 4
groups globally so the pipeline never drains at chunk/head boundaries.
Final normalization (outT / sums) and transpose on host.
"""

import os
from contextlib import ExitStack

import ml_dtypes
import numpy as np

_B, _H, _S, _D = 2, 16, 2048, 128
_NCORES = 8
_NH = (_B * _H) // _NCORES  # heads per core = 4
_QCH = 512
_NQC = _S // _QCH
_G = 3  # blocks per exp/mult group
_FSX = 0.25  # fraction of each chunk's key blocks whose sums go via PE
_PSX = 0  # (unused) Pool sums share
_PIPE = 4  # group-level software pipeline lag for PV/sums emission

_PROG_CACHE = {}
LAST_RESULTS = None
LAST_IN_MAPS = None
LAST_BUILD_KW = None


def _schedule(caps, G=_G, fsx=_FSX, psx=_PSX):
    """Static per-core schedule; shared by host packing and device build."""
    plan = []
    off = 0
    for h, cap in enumerate(caps):
        hplan = []
        for qc in range(_NQC):
            q_end = (qc + 1) * _QCH
            jb_hi = min((q_end + 127) // 128, cap)
            groups = []
            g0 = 0
            while g0 < jb_hi:
                Gp = min(G, jb_hi - g0)
                qo = max(0, g0 * 128 - qc * _QCH)
                nq = _QCH - qo
                groups.append({"g0": g0, "Gp": Gp, "qo": qo, "nq": nq, "off": off})
                off += Gp * nq
                g0 += Gp
            # fs rounded to a multiple of G: no group straddles the one-hot
            # boundary, so gsum sums need no memset (first group copy-inits)
            fs = G * int(round(fsx * jb_hi / G)) if fsx > 0 else 0
            fs = min(fs, jb_hi)
            hplan.append({"jb_hi": jb_hi, "fs": fs, "groups": groups})
        plan.append(hplan)
    return plan, off


def _build_program(caps, repeat=1, G=_G, fsx=_FSX, psx=_PSX, pipe=_PIPE, drop="", unroll=False, gsum=_GSUM):
    import contextlib

    import concourse.bacc as bacc
    import concourse.mybir as mybir
    import concourse.tile as tile

    caps = tuple(caps)
    NH, S, D, QCH, NQC = _NH, _S, _D, _QCH, _NQC
    NB = S // 128
    kof = [sum(caps[:h]) for h in range(NH + 1)]  # ragged key-block offsets
    f32 = mybir.dt.float32
    f16 = mybir.dt.float16
    bf16 = mybir.dt.bfloat16

    plan, CF = _schedule(caps, G, fsx, psx)
    use_r = any(ch["fs"] > 0 for hp in plan for ch in hp)

    nc = bacc.Bacc("TRN2", target_bir_lowering=False, debug=False)

    KBT = kof[NH]  # total key blocks across heads
    kt_d = nc.dram_tensor("kt", [128, KBT * 128], bf16, kind="ExternalInput").ap()
    qt_d = nc.dram_tensor("qt", [NH, 128, S], bf16, kind="ExternalInput").ap()
    v_d = nc.dram_tensor("v", [128, KBT, D], f16, kind="ExternalInput").ap()
    eb_d = nc.dram_tensor("eb", [128, CF], f16, kind="ExternalInput").ap()
    outT_d = nc.dram_tensor("outT", [NH, D, S], bf16, kind="ExternalOutput").ap()
    SW = G if gsum else 1  # sacc slot width
    sacc_d = nc.dram_tensor(
        "sacc", [NH, NQC, 128, SW, QCH], f16, kind="ExternalOutput"
    ).ap()
    if use_r:
        r_d = nc.dram_tensor("r", [NH, NQC, QCH], f32, kind="ExternalOutput").ap()
        oh_d = nc.dram_tensor("oh", [128, NQC * NQC], f16, kind="ExternalInput").ap()

    with tile.TileContext(nc) as tc, ExitStack() as ctx:
        const = ctx.enter_context(tc.tile_pool(name="const", bufs=1))
        ptp = ctx.enter_context(tc.tile_pool(name="ptp", bufs=2))
        ptmp = ctx.enter_context(tc.tile_pool(name="ptmp", bufs=pipe + 1 + (1 if "ptm6" in drop else 0)))
        saccp = ctx.enter_context(tc.tile_pool(name="saccp", bufs=2))
        obp = ctx.enter_context(tc.tile_pool(name="obp", bufs=1 if pipe >= 5 else 2))
        rbp = ctx.enter_context(tc.tile_pool(name="rbp", bufs=1))
        psum_s = ctx.enter_context(tc.tile_pool(name="psum_s", bufs=3 if G == 2 else 2, space="PSUM"))
        psum_o = ctx.enter_context(tc.tile_pool(name="psum_o", bufs=1, space="PSUM"))
        if use_r:
            psum_r = ctx.enter_context(tc.tile_pool(name="psum_r", bufs=1, space="PSUM"))

        # one-hot "ones" weights (host-provided): oh[:, qc*NQC+qc] = 1, so chunk
        # qc's sums land in psum partition row qc of a per-head [NQC, QCH] accum
        if use_r:
            oh_sb = const.tile([128, NQC * NQC], f16)
            nc.sync.dma_start(out=oh_sb[:], in_=oh_d[:])
            ohs = [oh_sb[:, qc * NQC : (qc + 1) * NQC] for qc in range(NQC)]

        kt_sb = const.tile([128, KBT * 128], bf16)
        qt_sb = const.tile([128, NH, S], bf16)
        v_sb = const.tile([128, KBT, D], f16)
        eb_sb = const.tile([128, CF], f16)
        nc.sync.dma_start(out=kt_sb[:], in_=kt_d[:])
        nc.sync.dma_start(out=v_sb[:], in_=v_d[:])
        for h in range(NH):
            nc.sync.dma_start(out=qt_sb[:, h, :], in_=qt_d[h])
        nch = 8
        step = (CF + nch - 1) // nch
        for i in range(nch):
            lo = i * step
            hi = min(CF, lo + step)
            if lo < hi:
                (nc.sync if i % 2 else nc.gpsimd).dma_start(
                    out=eb_sb[:, lo:hi], in_=eb_d[:, lo:hi]
                )

        loop_cm = (
            tc.For_i(0, repeat, 1)
            if (repeat > 1 and not unroll)
            else contextlib.nullcontext()
        )
        with loop_cm:
          for _rep in range(repeat if unroll else 1):
            # flat stage list: one entry per (head, chunk, group); the B-stage
            # (PV + sums) trails the A-stage (QK + exp + mult) by `pipe` slots
            # globally, so the cross-engine pipeline never drains at chunk or
            # head boundaries.
            stages = []
            head_ctx = {}
            chunk_ctx = {}
            for h in range(NH):
                fsqc = [q for q in range(NQC) if plan[h][q]["fs"] > 0]
                head_ctx[h] = {
                    "r_ps": None,
                    "fs_first": fsqc[0] if fsqc else -1,
                    "fs_last": fsqc[-1] if fsqc else -1,
                }
                for qc in range(NQC):
                    chunk = plan[h][qc]
                    ck = {
                        "h": h,
                        "qc": qc,
                        "jb_hi": chunk["jb_hi"],
                        "fs": chunk["fs"],
                        "o_ps": None,
                        "sacc": None,
                    }
                    chunk_ctx[(h, qc)] = ck
                    ngrp = len(chunk["groups"])
                    for idx, g in enumerate(chunk["groups"]):
                        stages.append((ck, g, idx, ngrp))

            stash = {}
            for gi in range(len(stages) + pipe):
                if gi < len(stages):
                    ck, g, _idx, _ngrp = stages[gi]
                    h, qc = ck["h"], ck["qc"]
                    g0, Gp, qo, nq, off = (
                        g["g0"], g["Gp"], g["qo"], g["nq"], g["off"],
                    )
                    s3 = psum_s.tile([128, G, QCH], f32)
                    for i in range(Gp):
                        jb = g0 + i
                        nc.tensor.matmul(
                            s3[:, i, qo:],
                            lhsT=kt_sb[
                                :, (kof[h] + jb) * 128 : (kof[h] + jb + 1) * 128
                            ],
                            rhs=qt_sb[:, h, qc * QCH + qo : (qc + 1) * QCH],
                            start=True,
                            stop=True,
                        )
                    pt3 = ptp.tile([128, G, QCH], f16)
                    nc.scalar.activation(
                        pt3[:, :Gp, qo:],
                        s3[:, :Gp, qo:],
                        mybir.ActivationFunctionType.Exp,
                    )
                    ptm3 = ptmp.tile([128, G, QCH], f16)
                    ebv = eb_sb[:, off : off + Gp * nq].rearrange(
                        "p (g n) -> p g n", g=Gp
                    )
                    if "mult" not in drop:
                        nc.vector.tensor_mul(ptm3[:, :Gp, qo:], pt3[:, :Gp, qo:], ebv)
                    else:
                        ptm3 = pt3
                    stash[gi] = ptm3
                bi = gi - pipe
                if 0 <= bi < len(stages):
                    ck, g, idx, ngrp = stages[bi]
                    last = idx == ngrp - 1
                    h, qc = ck["h"], ck["qc"]
                    jb_hi, fs = ck["jb_hi"], ck["fs"]
                    g0, Gp, qo = g["g0"], g["Gp"], g["qo"]
                    ptm3 = stash.pop(bi)
                    if ck["o_ps"] is None:
                        ck["o_ps"] = psum_o.tile([128, QCH], f32, name="o_ps")
                        if fs < jb_hi:
                            ck["sacc"] = saccp.tile([128, SW, QCH], f16, name="sacc")
                            if gsum:
                                ck["sacc_init"] = False
                                if min(G, jb_hi - fs) < G:
                                    # slots never touched by the copy-init
                                    nc.gpsimd.memset(
                                        ck["sacc"][:, min(G, jb_hi - fs):, :], 0.0
                                    )
                    if fs > 0 and head_ctx[h]["r_ps"] is None:
                        head_ctx[h]["r_ps"] = psum_r.tile(
                            [NQC, QCH], f32, name="r_ps"
                        )
                    o_ps, sacc = ck["o_ps"], ck["sacc"]
                    r_ps = head_ctx[h]["r_ps"]
                    for i in range(Gp):
                        jb = g0 + i
                        qb = max(qo, jb * 128 - qc * QCH)  # per-block trim
                        if "pv" not in drop:
                            nc.tensor.matmul(
                                o_ps[:, qb:],
                                lhsT=v_sb[:, kof[h] + jb, :],
                                rhs=ptm3[:, i, qb:],
                                start=(jb == 0),
                                stop=(jb == jb_hi - 1),
                            )
                        elif jb == 0:
                            nc.tensor.matmul(
                                o_ps[:, :],
                                lhsT=v_sb[:, kof[h] + jb, :],
                                rhs=ptm3[:, i, :],
                                start=True,
                                stop=True,
                            )
                        if jb < fs:
                            nc.tensor.matmul(
                                r_ps[:, qb:],
                                lhsT=ohs[qc],
                                rhs=ptm3[:, i, qb:],
                                start=(qc == head_ctx[h]["fs_first"] and jb == 0),
                                stop=(qc == head_ctx[h]["fs_last"] and jb == fs - 1),
                                skip_group_check=True,
                            )
                        elif "sacc" in drop:
                            pass
                        elif gsum:
                            pass  # fs is a multiple of G: no straddle groups
                        elif jb == fs:
                            nc.vector.tensor_copy(sacc[:, 0, qb:], ptm3[:, i, qb:])
                            if qb > 0:
                                nc.gpsimd.memset(sacc[:, 0, :qb], 0.0)
                        else:
                            nc.vector.tensor_add(
                                sacc[:, 0, qb:], sacc[:, 0, qb:], ptm3[:, i, qb:]
                            )
                    if gsum and g0 >= fs and "sacc" not in drop:
                        # whole group accumulated in one DVE op (slot per block);
                        # first group per chunk copy-inits (qo == 0 there)
                        if not ck["sacc_init"]:
                            nc.vector.tensor_copy(
                                sacc[:, :Gp, qo:], ptm3[:, :Gp, qo:]
                            )
                            ck["sacc_init"] = True
                        else:
                            nc.vector.tensor_add(
                                sacc[:, :Gp, qo:], sacc[:, :Gp, qo:], ptm3[:, :Gp, qo:]
                            )
                    lg_qo = plan[h][qc]["groups"][-1]["qo"]
                    dve_drain = "obdve" in drop or (
                        "obact" not in drop and (h * NQC + qc) % 2 == 0
                    )
                    if ngrp >= 2 and idx == ngrp - 2 and lg_qo > 0:
                        # cols [0, lg_qo) are final: the last group's PVs only
                        # write [lg_qo:], so drain the prefix one slot early,
                        # leaving just the tail on the chunk-boundary WAR path
                        ck["ob"] = obp.tile([128, QCH], bf16, name="ob")
                        if dve_drain:
                            nc.vector.tensor_copy(ck["ob"][:, :lg_qo], o_ps[:, :lg_qo])
                        else:
                            nc.scalar.copy(ck["ob"][:, :lg_qo], o_ps[:, :lg_qo])
                        ck["early"] = lg_qo
                    if last:
                        ob = ck.get("ob")
                        if ob is None:
                            ob = obp.tile([128, QCH], bf16, name="ob")
                        lo = ck.get("early", 0)
                        if dve_drain:
                            nc.vector.tensor_copy(ob[:, lo:], o_ps[:, lo:])
                        else:
                            nc.scalar.copy(ob[:, lo:], o_ps[:, lo:])
                        nc.gpsimd.dma_start(
                            out=outT_d[h, :, qc * QCH : (qc + 1) * QCH], in_=ob[:]
                        )
                        if sacc is not None and "sacc" not in drop:
                            nc.gpsimd.dma_start(out=sacc_d[h, qc], in_=sacc[:])
                        if fs > 0 and qc == NQC - 1:
                            rb_h = rbp.tile([NQC, QCH], f32)
                            nc.vector.tensor_copy(rb_h[:], r_ps[:])
                            nc.sync.dma_start(out=r_d[h], in_=rb_h[:])
                            head_ctx[h]["r_ps"] = None

    nc.compile()
    return nc


def _pack_ebias(eb_masked, caps, G=_G, fsx=_FSX, psx=_PSX):
    """eb_masked: [NH, S(j), S(q)] f32 (exp(bias) with masks folded as 0).
    Returns [128, CF] fp16 ragged-packed per the schedule."""
    plan, CF = _schedule(caps, G, fsx, psx)
    out = np.zeros((128, CF), dtype=np.float16)
    for h, hplan in enumerate(plan):
        for qc, chunk in enumerate(hplan):
            for g in chunk["groups"]:
                g0, Gp, qo, nq, off = g["g0"], g["Gp"], g["qo"], g["nq"], g["off"]
                for i in range(Gp):
                    jb = g0 + i
                    blk = eb_masked[
                        h,
                        jb * 128 : (jb + 1) * 128,
                        qc * _QCH + qo : (qc + 1) * _QCH,
                    ]
                    out[:, off + i * nq : off + (i + 1) * nq] = blk.astype(np.float16)
    return out


def _run_multicore(in_maps, caps):
    global LAST_RESULTS, LAST_IN_MAPS, LAST_BUILD_KW
    from concourse.bass_utils import run_bass_kernel_spmd

    key = (tuple(caps), _G, _FSX, _PSX, _PIPE, _GSUM)
    if key not in _PROG_CACHE:
        _PROG_CACHE[key] = _build_program(caps)
    nc = _PROG_CACHE[key]
    LAST_IN_MAPS = in_maps
    LAST_BUILD_KW = {"caps": tuple(caps), "G": _G, "fsx": _FSX, "psx": _PSX, "pipe": _PIPE, "gsum": _GSUM}
    res = run_bass_kernel_spmd(nc, in_maps, core_ids=list(range(len(in_maps))))
    LAST_RESULTS = res
    return res.results


def kernel(q, k, v, mask, attn_bias, offset):
    B, H, S, D = _B, _H, _S, _D
    q = np.asarray(q, dtype=np.float32)
    k = np.asarray(k, dtype=np.float32)
    v = np.asarray(v, dtype=np.float32)
    mask = np.asarray(mask).astype(bool)
    attn_bias = np.asarray(attn_bias, dtype=np.float32)
    off = int(np.asarray(offset))

    scale = np.float32(D**-0.5)
    valid = mask[:, 0, 0, :]  # [B, S]

    caps_b = []
    for b in range(B):
        idx = np.nonzero(valid[b])[0]
        lv = (int(idx[-1]) + 1) if len(idx) else 1
        caps_b.append(max(1, (lv + 127) // 128))

    # ebias[h][j, i] = exp(attn_bias[h, i, j]), causal mask j >= i+1-off -> 0
    jj = np.arange(S)[:, None]
    ii = np.arange(S)[None, :]
    keep_causal = (jj < (ii + 1 - off)).astype(np.float32)  # [j, i]
    ebias = np.exp(attn_bias).transpose(0, 2, 1) * keep_causal[None]

    core_pairs = [
        [(0, 2 * c), (0, 2 * c + 1), (1, 2 * c), (1, 2 * c + 1)] for c in range(_NCORES)
    ]
    caps = tuple(caps_b[b] for (b, _h) in core_pairs[0])

    NB = S // 128
    bf = ml_dtypes.bfloat16
    in_maps = []
    for c in range(_NCORES):
        pairs = core_pairs[c]
        kt = np.concatenate(
            [(k[b, h][: caps_b[b] * 128] * scale).T for (b, h) in pairs], axis=1
        ).astype(bf)
        qt = np.stack([q[b, h].T for (b, h) in pairs]).astype(bf)
        vv = np.concatenate(
            [
                v[b, h][: caps_b[b] * 128].reshape(caps_b[b], 128, D).transpose(1, 0, 2)
                for (b, h) in pairs
            ],
            axis=1,
        ).astype(np.float16)
        ebm = np.stack(
            [ebias[h] * valid[b][:, None].astype(np.float32) for (b, h) in pairs]
        )
        oh = np.zeros((128, _NQC * _NQC), np.float16)
        for qc in range(_NQC):
            oh[:, qc * _NQC + qc] = 1.0
        in_maps.append(
            {
                "kt": np.ascontiguousarray(kt),
                "qt": np.ascontiguousarray(qt),
                "v": np.ascontiguousarray(vv),
                "eb": _pack_ebias(ebm, caps),
                "oh": oh,
            }
        )

    results = _run_multicore(in_maps, caps)

    out = np.empty((B, H, S, D), dtype=np.float32)
    for c in range(_NCORES):
        res = results[c]
        outT = res["outT"].astype(np.float32)  # [NH, D, S]
        sums = res["sacc"].astype(np.float32).sum(axis=(2, 3)).reshape(_NH, S)
        if "r" in res:
            sums = sums + res["r"].reshape(_NH, S)
        for i, (b, h) in enumerate(core_pairs[c]):
            out[b, h] = (outT[i] / sums[i][None, :]).T
    return out



# revision 23
# speedup vs baseline: 2.9638x; 1.5338x over previous
"""Trainium2 Bass kernel for causal attention with additive bias + key padding mask.

Problem: B=2, H=16, S=2048, D=128 (fp32), attn_bias [H,S,S], mask [B,1,1,S], offset=0.

Sharding: 32 (b,h) pairs across 8 cores, mixed-batch: core c gets heads (2c, 2c+1)
of BOTH batches so every core sees the same mix of key-length caps.

Device math (per core, scores kept transposed: s_T[j, q], so no transposes):
  s_T = KT_blk^T @ QT_chunk          (PE, bf16, psum f32)
  pt  = exp(s_T)                     (ACT, psum -> sbuf fp16, 3-block grouped)
  ptm = pt * ebias                   (DVE fp16 2x; ebias = exp(bias) with causal +
                                      key-pad masks folded as exact zeros, fp16,
                                      packed ragged and fully SBUF-resident)
  out_T[d,q] += V_blk^T @ ptm        (PE, fp16 rhs)
  sums hybrid: leading fs blocks per chunk via PE one-hot-ones matmul into a
        per-head [NQC,512] psum (one DVE drain per head); remaining blocks
        accumulated into a fp16 sacc tile on DVE (copy-init). Host combines.
The o_ps drain (psum -> bf16 staging) alternates between the ACT and DVE
engines per chunk to balance queue load; the B-stage trails the A-stage by 4
groups globally so the pipeline never drains at chunk/head boundaries.
Final normalization (outT / sums) and transpose on host.
"""

import os
from contextlib import ExitStack

import ml_dtypes
import numpy as np

_B, _H, _S, _D = 2, 16, 2048, 128
_NCORES = 8
_NH = (_B * _H) // _NCORES  # heads per core = 4
_QCH = 512
_NQC = _S // _QCH
_G = 3  # blocks per exp/mult group
_FSX = 0.25  # fraction of each chunk's key blocks whose sums go via PE
_PSX = 0  # (unused) Pool sums share
_PIPE = 4  # group-level software pipeline lag for PV/sums emission

_PROG_CACHE = {}
LAST_RESULTS = None
LAST_IN_MAPS = None
LAST_BUILD_KW = None


def _schedule(caps, G=_G, fsx=_FSX, psx=_PSX):
    """Static per-core schedule; shared by host packing and device build."""
    plan = []
    off = 0
    for h, cap in enumerate(caps):
        hplan = []
        for qc in range(_NQC):
            q_end = (qc + 1) * _QCH
            jb_hi = min((q_end + 127) // 128, cap)
            groups = []
            g0 = 0
            while g0 < jb_hi:
                Gp = min(G, jb_hi - g0)
                qo = max(0, g0 * 128 - qc * _QCH)
                nq = _QCH - qo
                groups.append({"g0": g0, "Gp": Gp, "qo": qo, "nq": nq, "off": off})
                off += Gp * nq
                g0 += Gp
            # fs rounded to a multiple of G: no group straddles the one-hot
            # boundary, so gsum sums need no memset (first group copy-inits)
            fs = G * int(round(fsx * jb_hi / G)) if fsx > 0 else 0
            fs = min(fs, jb_hi)
            hplan.append({"jb_hi": jb_hi, "fs": fs, "groups": groups})
        plan.append(hplan)
    return plan, off


def _build_program(caps, repeat=1, G=_G, fsx=_FSX, psx=_PSX, pipe=_PIPE, drop="", unroll=False, gsum=_GSUM):
    import contextlib

    import concourse.bacc as bacc
    import concourse.mybir as mybir
    import concourse.tile as tile

    caps = tuple(caps)
    NH, S, D, QCH, NQC = _NH, _S, _D, _QCH, _NQC
    NB = S // 128
    kof = [sum(caps[:h]) for h in range(NH + 1)]  # ragged key-block offsets
    f32 = mybir.dt.float32
    f16 = mybir.dt.float16
    bf16 = mybir.dt.bfloat16

    plan, CF = _schedule(caps, G, fsx, psx)
    use_r = any(ch["fs"] > 0 for hp in plan for ch in hp)

    nc = bacc.Bacc("TRN2", target_bir_lowering=False, debug=False)

    KBT = kof[NH]  # total key blocks across heads
    kt_d = nc.dram_tensor("kt", [128, KBT * 128], bf16, kind="ExternalInput").ap()
    qt_d = nc.dram_tensor("qt", [NH, 128, S], bf16, kind="ExternalInput").ap()
    v_d = nc.dram_tensor("v", [128, KBT, D], f16, kind="ExternalInput").ap()
    eb_d = nc.dram_tensor("eb", [128, CF], f16, kind="ExternalInput").ap()
    outT_d = nc.dram_tensor("outT", [NH, D, S], bf16, kind="ExternalOutput").ap()
    SW = G if gsum else 1  # sacc slot width
    sacc_d = nc.dram_tensor(
        "sacc", [NH, NQC, 128, SW, QCH], f16, kind="ExternalOutput"
    ).ap()
    if use_r:
        r_d = nc.dram_tensor("r", [NH, NQC, QCH], f32, kind="ExternalOutput").ap()
        oh_d = nc.dram_tensor("oh", [128, NQC * NQC], f16, kind="ExternalInput").ap()

    with tile.TileContext(nc) as tc, ExitStack() as ctx:
        const = ctx.enter_context(tc.tile_pool(name="const", bufs=1))
        ptp = ctx.enter_context(tc.tile_pool(name="ptp", bufs=2))
        ptmp = ctx.enter_context(tc.tile_pool(name="ptmp", bufs=pipe + 1 + (1 if "ptm6" in drop else 0)))
        saccp = ctx.enter_context(tc.tile_pool(name="saccp", bufs=2))
        obp = ctx.enter_context(tc.tile_pool(name="obp", bufs=1 if pipe >= 5 else 2))
        rbp = ctx.enter_context(tc.tile_pool(name="rbp", bufs=1))
        psum_s = ctx.enter_context(tc.tile_pool(name="psum_s", bufs=3 if G == 2 else 2, space="PSUM"))
        psum_o = ctx.enter_context(tc.tile_pool(name="psum_o", bufs=1, space="PSUM"))
        if use_r:
            psum_r = ctx.enter_context(tc.tile_pool(name="psum_r", bufs=1, space="PSUM"))

        # one-hot "ones" weights (host-provided): oh[:, qc*NQC+qc] = 1, so chunk
        # qc's sums land in psum partition row qc of a per-head [NQC, QCH] accum
        if use_r:
            oh_sb = const.tile([128, NQC * NQC], f16)
            nc.sync.dma_start(out=oh_sb[:], in_=oh_d[:])
            ohs = [oh_sb[:, qc * NQC : (qc + 1) * NQC] for qc in range(NQC)]

        kt_sb = const.tile([128, KBT * 128], bf16)
        qt_sb = const.tile([128, NH, S], bf16)
        v_sb = const.tile([128, KBT, D], f16)
        eb_sb = const.tile([128, CF], f16)
        nc.sync.dma_start(out=kt_sb[:], in_=kt_d[:])
        nc.sync.dma_start(out=v_sb[:], in_=v_d[:])
        for h in range(NH):
            nc.sync.dma_start(out=qt_sb[:, h, :], in_=qt_d[h])
        nch = 8
        step = (CF + nch - 1) // nch
        for i in range(nch):
            lo = i * step
            hi = min(CF, lo + step)
            if lo < hi:
                (nc.sync if i % 2 else nc.gpsimd).dma_start(
                    out=eb_sb[:, lo:hi], in_=eb_d[:, lo:hi]
                )

        loop_cm = (
            tc.For_i(0, repeat, 1)
            if (repeat > 1 and not unroll)
            else contextlib.nullcontext()
        )
        with loop_cm:
          for _rep in range(repeat if unroll else 1):
            # flat stage list: one entry per (head, chunk, group); the B-stage
            # (PV + sums) trails the A-stage (QK + exp + mult) by `pipe` slots
            # globally, so the cross-engine pipeline never drains at chunk or
            # head boundaries.
            stages = []
            head_ctx = {}
            chunk_ctx = {}
            for h in range(NH):
                fsqc = [q for q in range(NQC) if plan[h][q]["fs"] > 0]
                head_ctx[h] = {
                    "r_ps": None,
                    "fs_first": fsqc[0] if fsqc else -1,
                    "fs_last": fsqc[-1] if fsqc else -1,
                }
                for qc in range(NQC):
                    chunk = plan[h][qc]
                    ck = {
                        "h": h,
                        "qc": qc,
                        "jb_hi": chunk["jb_hi"],
                        "fs": chunk["fs"],
                        "o_ps": None,
                        "sacc": None,
                    }
                    chunk_ctx[(h, qc)] = ck
                    ngrp = len(chunk["groups"])
                    for idx, g in enumerate(chunk["groups"]):
                        stages.append((ck, g, idx, ngrp))

            stash = {}
            for gi in range(len(stages) + pipe):
                if gi < len(stages):
                    ck, g, _idx, _ngrp = stages[gi]
                    h, qc = ck["h"], ck["qc"]
                    g0, Gp, qo, nq, off = (
                        g["g0"], g["Gp"], g["qo"], g["nq"], g["off"],
                    )
                    s3 = psum_s.tile([128, G, QCH], f32)
                    for i in range(Gp):
                        jb = g0 + i
                        nc.tensor.matmul(
                            s3[:, i, qo:],
                            lhsT=kt_sb[
                                :, (kof[h] + jb) * 128 : (kof[h] + jb + 1) * 128
                            ],
                            rhs=qt_sb[:, h, qc * QCH + qo : (qc + 1) * QCH],
                            start=True,
                            stop=True,
                        )
                    pt3 = ptp.tile([128, G, QCH], f16)
                    nc.scalar.activation(
                        pt3[:, :Gp, qo:],
                        s3[:, :Gp, qo:],
                        mybir.ActivationFunctionType.Exp,
                    )
                    ptm3 = ptmp.tile([128, G, QCH], f16)
                    ebv = eb_sb[:, off : off + Gp * nq].rearrange(
                        "p (g n) -> p g n", g=Gp
                    )
                    if "mult" not in drop:
                        nc.vector.tensor_mul(ptm3[:, :Gp, qo:], pt3[:, :Gp, qo:], ebv)
                    else:
                        ptm3 = pt3
                    stash[gi] = ptm3
                bi = gi - pipe
                if 0 <= bi < len(stages):
                    ck, g, idx, ngrp = stages[bi]
                    last = idx == ngrp - 1
                    h, qc = ck["h"], ck["qc"]
                    jb_hi, fs = ck["jb_hi"], ck["fs"]
                    g0, Gp, qo = g["g0"], g["Gp"], g["qo"]
                    ptm3 = stash.pop(bi)
                    if ck["o_ps"] is None:
                        ck["o_ps"] = psum_o.tile([128, QCH], f32, name="o_ps")
                        if fs < jb_hi:
                            ck["sacc"] = saccp.tile([128, SW, QCH], f16, name="sacc")
                            if gsum:
                                ck["sacc_init"] = False
                                if min(G, jb_hi - fs) < G:
                                    # slots never touched by the copy-init
                                    nc.gpsimd.memset(
                                        ck["sacc"][:, min(G, jb_hi - fs):, :], 0.0
                                    )
                    if fs > 0 and head_ctx[h]["r_ps"] is None:
                        head_ctx[h]["r_ps"] = psum_r.tile(
                            [NQC, QCH], f32, name="r_ps"
                        )
                    o_ps, sacc = ck["o_ps"], ck["sacc"]
                    r_ps = head_ctx[h]["r_ps"]
                    for i in range(Gp):
                        jb = g0 + i
                        qb = max(qo, jb * 128 - qc * QCH)  # per-block trim
                        if "pv" not in drop:
                            nc.tensor.matmul(
                                o_ps[:, qb:],
                                lhsT=v_sb[:, kof[h] + jb, :],
                                rhs=ptm3[:, i, qb:],
                                start=(jb == 0),
                                stop=(jb == jb_hi - 1),
                            )
                        elif jb == 0:
                            nc.tensor.matmul(
                                o_ps[:, :],
                                lhsT=v_sb[:, kof[h] + jb, :],
                                rhs=ptm3[:, i, :],
                                start=True,
                                stop=True,
                            )
                        if jb < fs:
                            nc.tensor.matmul(
                                r_ps[:, qb:],
                                lhsT=ohs[qc],
                                rhs=ptm3[:, i, qb:],
                                start=(qc == head_ctx[h]["fs_first"] and jb == 0),
                                stop=(qc == head_ctx[h]["fs_last"] and jb == fs - 1),
                                skip_group_check=True,
                            )
                        elif "sacc" in drop:
                            pass
                        elif gsum:
                            pass  # fs is a multiple of G: no straddle groups
                        elif jb == fs:
                            nc.vector.tensor_copy(sacc[:, 0, qb:], ptm3[:, i, qb:])
                            if qb > 0:
                                nc.gpsimd.memset(sacc[:, 0, :qb], 0.0)
                        else:
                            nc.vector.tensor_add(
                                sacc[:, 0, qb:], sacc[:, 0, qb:], ptm3[:, i, qb:]
                            )
                    if gsum and g0 >= fs and "sacc" not in drop:
                        # whole group accumulated in one DVE op (slot per block);
                        # first group per chunk copy-inits (qo == 0 there)
                        if not ck["sacc_init"]:
                            nc.vector.tensor_copy(
                                sacc[:, :Gp, qo:], ptm3[:, :Gp, qo:]
                            )
                            ck["sacc_init"] = True
                        else:
                            nc.vector.tensor_add(
                                sacc[:, :Gp, qo:], sacc[:, :Gp, qo:], ptm3[:, :Gp, qo:]
                            )
                    lg_qo = plan[h][qc]["groups"][-1]["qo"]
                    dve_drain = "obdve" in drop  # all drains on ACT:
                    # at chunk boundaries DVE is busy with the gsum add and
                    # the next chunk's sacc copy-init; ACT runs pipe-ahead
                    if ngrp >= 2 and idx == ngrp - 2 and lg_qo > 0:
                        # cols [0, lg_qo) are final: the last group's PVs only
                        # write [lg_qo:], so drain the prefix one slot early,
                        # leaving just the tail on the chunk-boundary WAR path
                        ck["ob"] = obp.tile([128, QCH], bf16, name="ob")
                        if dve_drain:
                            nc.vector.tensor_copy(ck["ob"][:, :lg_qo], o_ps[:, :lg_qo])
                        else:
                            nc.scalar.copy(ck["ob"][:, :lg_qo], o_ps[:, :lg_qo])
                        ck["early"] = lg_qo
                    if last:
                        ob = ck.get("ob")
                        if ob is None:
                            ob = obp.tile([128, QCH], bf16, name="ob")
                        lo = ck.get("early", 0)
                        if dve_drain:
                            nc.vector.tensor_copy(ob[:, lo:], o_ps[:, lo:])
                        else:
                            nc.scalar.copy(ob[:, lo:], o_ps[:, lo:])
                        nc.gpsimd.dma_start(
                            out=outT_d[h, :, qc * QCH : (qc + 1) * QCH], in_=ob[:]
                        )
                        if sacc is not None and "sacc" not in drop:
                            nc.gpsimd.dma_start(out=sacc_d[h, qc], in_=sacc[:])
                        if fs > 0 and qc == NQC - 1:
                            rb_h = rbp.tile([NQC, QCH], f32)
                            nc.vector.tensor_copy(rb_h[:], r_ps[:])
                            nc.sync.dma_start(out=r_d[h], in_=rb_h[:])
                            head_ctx[h]["r_ps"] = None

    nc.compile()
    return nc


def _pack_ebias(eb_masked, caps, G=_G, fsx=_FSX, psx=_PSX):
    """eb_masked: [NH, S(j), S(q)] f32 (exp(bias) with masks folded as 0).
    Returns [128, CF] fp16 ragged-packed per the schedule."""
    plan, CF = _schedule(caps, G, fsx, psx)
    out = np.zeros((128, CF), dtype=np.float16)
    for h, hplan in enumerate(plan):
        for qc, chunk in enumerate(hplan):
            for g in chunk["groups"]:
                g0, Gp, qo, nq, off = g["g0"], g["Gp"], g["qo"], g["nq"], g["off"]
                for i in range(Gp):
                    jb = g0 + i
                    blk = eb_masked[
                        h,
                        jb * 128 : (jb + 1) * 128,
                        qc * _QCH + qo : (qc + 1) * _QCH,
                    ]
                    out[:, off + i * nq : off + (i + 1) * nq] = blk.astype(np.float16)
    return out


def _run_multicore(in_maps, caps):
    global LAST_RESULTS, LAST_IN_MAPS, LAST_BUILD_KW
    from concourse.bass_utils import run_bass_kernel_spmd

    key = (tuple(caps), _G, _FSX, _PSX, _PIPE, _GSUM)
    if key not in _PROG_CACHE:
        _PROG_CACHE[key] = _build_program(caps)
    nc = _PROG_CACHE[key]
    LAST_IN_MAPS = in_maps
    LAST_BUILD_KW = {"caps": tuple(caps), "G": _G, "fsx": _FSX, "psx": _PSX, "pipe": _PIPE, "gsum": _GSUM}
    res = run_bass_kernel_spmd(nc, in_maps, core_ids=list(range(len(in_maps))))
    LAST_RESULTS = res
    return res.results


def kernel(q, k, v, mask, attn_bias, offset):
    B, H, S, D = _B, _H, _S, _D
    q = np.asarray(q, dtype=np.float32)
    k = np.asarray(k, dtype=np.float32)
    v = np.asarray(v, dtype=np.float32)
    mask = np.asarray(mask).astype(bool)
    attn_bias = np.asarray(attn_bias, dtype=np.float32)
    off = int(np.asarray(offset))

    scale = np.float32(D**-0.5)
    valid = mask[:, 0, 0, :]  # [B, S]

    caps_b = []
    for b in range(B):
        idx = np.nonzero(valid[b])[0]
        lv = (int(idx[-1]) + 1) if len(idx) else 1
        caps_b.append(max(1, (lv + 127) // 128))

    # ebias[h][j, i] = exp(attn_bias[h, i, j]), causal mask j >= i+1-off -> 0
    jj = np.arange(S)[:, None]
    ii = np.arange(S)[None, :]
    keep_causal = (jj < (ii + 1 - off)).astype(np.float32)  # [j, i]
    ebias = np.exp(attn_bias).transpose(0, 2, 1) * keep_causal[None]

    core_pairs = [
        [(0, 2 * c), (0, 2 * c + 1), (1, 2 * c), (1, 2 * c + 1)] for c in range(_NCORES)
    ]
    caps = tuple(caps_b[b] for (b, _h) in core_pairs[0])

    NB = S // 128
    bf = ml_dtypes.bfloat16
    in_maps = []
    for c in range(_NCORES):
        pairs = core_pairs[c]
        kt = np.concatenate(
            [(k[b, h][: caps_b[b] * 128] * scale).T for (b, h) in pairs], axis=1
        ).astype(bf)
        qt = np.stack([q[b, h].T for (b, h) in pairs]).astype(bf)
        vv = np.concatenate(
            [
                v[b, h][: caps_b[b] * 128].reshape(caps_b[b], 128, D).transpose(1, 0, 2)
                for (b, h) in pairs
            ],
            axis=1,
        ).astype(np.float16)
        ebm = np.stack(
            [ebias[h] * valid[b][:, None].astype(np.float32) for (b, h) in pairs]
        )
        oh = np.zeros((128, _NQC * _NQC), np.float16)
        for qc in range(_NQC):
            oh[:, qc * _NQC + qc] = 1.0
        in_maps.append(
            {
                "kt": np.ascontiguousarray(kt),
                "qt": np.ascontiguousarray(qt),
                "v": np.ascontiguousarray(vv),
                "eb": _pack_ebias(ebm, caps),
                "oh": oh,
            }
        )

    results = _run_multicore(in_maps, caps)

    out = np.empty((B, H, S, D), dtype=np.float32)
    for c in range(_NCORES):
        res = results[c]
        outT = res["outT"].astype(np.float32)  # [NH, D, S]
        sums = res["sacc"].astype(np.float32).sum(axis=(2, 3)).reshape(_NH, S)
        if "r" in res:
            sums = sums + res["r"].reshape(_NH, S)
        for i, (b, h) in enumerate(core_pairs[c]):
            out[b, h] = (outT[i] / sums[i][None, :]).T
    return out

